# revision 26
# baseline (speedup 1.0000x reference)
"""AssistedExcitation Trainium2 kernel.

out[b,c,h,w] = x[b,c,h,w] + bbox_mask[b,h,w] * mean_c(x[b,:,h,w])

Data-parallel over 8 NeuronCores: 2 images per core, no collectives.
HBM I/O in bf16 (rel-err budget 2e-2 >> bf16 rounding ~3e-3): halves
DMA traffic vs f32 -> ~94us/core roofline at 358 GB/s.

Per core, per [256, 4096] chunk (channel halves A/B on partitions):
  - channel sums via matmul with a 1/256 bf16 column into [1,1024]
    PSUM pair tiles,
  - DVE mul with the flat [1,HW] bf16 bbox mask -> masked means (ad),
  - K=1 broadcast matmuls spread ad across 128 partitions (PSUM),
    grouped after all sums so the PE runs long same-stationary streaks
    (TRN2 PE p-state: full 2.4 GHz only after ~3us continuous busy),
  - ACT copies PSUM->SBUF bf16, DVE does two fused 4096-wide bf16 adds
    (all-SBUF bf16 step-1 => 2x DVE packing mode),
  - chunk pipeline is software-staggered: adds/stores of chunk c-1 are
    emitted after the front half of chunk c, keeping every in-order
    engine queue free of cross-engine round-trip stalls.
Preamble (box rasterization) reads one packed [128,146] const DMA on
the scalar ring so it never queues behind 1 MiB x loads on sync.
"""

import sys

sys.path.insert(0, "/opt/trn_rl_repo")

import ml_dtypes
import numpy as np

import concourse.bacc as bacc
import concourse.bass as bass
import concourse.mybir as mybir
import concourse.tile as tile
from concourse import bass_utils

# Problem constants (hardcoded per harness contract)
B, C, H, W = 16, 256, 128, 128
N_BOX = 320
N_CORES = 8
B_SHARD = B // N_CORES  # 2 images per core
HW = H * W  # 16384
P = 128  # partitions
CHUNK = 4096  # free-dim elements per x tile (32 rows of the image)
N_CHUNK = HW // CHUNK  # 4
SUB = 512  # matmul moving free-dim (one PSUM bank of f32)
PAIR = 2 * SUB  # 1024: one [1, PAIR] PSUM sum tile = 2 banks
N_PAIR = CHUNK // PAIR  # 4
NBOX_PAD = 384  # 320 boxes padded to 3 tiles of 128
N_BOX_TILES = NBOX_PAD // P  # 3
ALPHA = 1.0
# packed const layout: [iota(128) | int box edges x1,x2,y1,y2 (3*4) | sel(3*2)]
CONST_COLS = P + 4 * N_BOX_TILES + 2 * N_BOX_TILES  # 146

F32 = mybir.dt.float32
BF16 = mybir.dt.bfloat16


def build_nc():
    """Build the per-core Bass graph (SPMD: same graph on all 8 cores)."""
    nc = bacc.Bacc(None, target_bir_lowering=False)

    x = nc.declare_dram_parameter("x", [B_SHARD, C, HW], BF16, isOutput=False)
    consts = nc.declare_dram_parameter("consts", [P, CONST_COLS], F32, isOutput=False)
    out = nc.declare_dram_parameter("out", [B_SHARD, C, HW], BF16, isOutput=True)

    with tile.TileContext(nc) as tc:
        with (
            tc.tile_pool(name="const", bufs=1) as constp,
            tc.tile_pool(name="boxp", bufs=1) as boxp,
            tc.tile_pool(name="maskp", bufs=1) as maskp,
            tc.tile_pool(name="xp", bufs=4) as xp,
            tc.tile_pool(name="outp", bufs=2) as outp,
            tc.tile_pool(name="pbsp", bufs=2) as pbsp,
            tc.tile_pool(name="adp", bufs=2) as adp,
            tc.tile_pool(name="smallp", bufs=2) as smallp,
            tc.tile_pool(name="ps_s", bufs=2, space=bass.MemorySpace.PSUM) as ps_s,
            tc.tile_pool(name="ps_b", bufs=3, space=bass.MemorySpace.PSUM) as ps_b,
            tc.tile_pool(name="ps_m", bufs=1, space=bass.MemorySpace.PSUM) as ps_m,
        ):
            # --- constants: one small DMA on the scalar ring ---
            cst = constp.tile([P, CONST_COLS], F32)
            nc.scalar.dma_start(cst[:], consts[:])
            iota_f = cst[:, 0:P]
            wsum = constp.tile([P, 1], BF16)  # 1/C column -> channel mean
            nc.vector.memset(wsum[:], ALPHA / C)
            ones1 = constp.tile([1, P], BF16)  # K=1 broadcast row
            nc.vector.memset(ones1[:], 1.0)
            # warm the Pool tensor_tensor ucode now: its first call pays a
            # ~6us IRAM load, which must not land on the steady-state path
            warm = constp.tile([1, 4], BF16)
            nc.gpsimd.memset(warm[:], 0.0)
            nc.gpsimd.tensor_add(warm[:], warm[:], warm[:])

            # --- box rasterization (tiny): integer box edges + validity are
            # host-precomputed (exact reference trunc/clamp semantics;
            # invalid boxes get x1=1,x2=0 -> empty). Device only compares:
            # cols[n,w] = (w >= x1) & (w <= x2).
            rows_sel = [[None] * N_BOX_TILES for _ in range(B_SHARD)]
            cols_val = [None] * N_BOX_TILES
            rows_raw, sel_tiles = [], []
            for t in range(N_BOX_TILES):
                ed = cst[:, P + 4 * t : P + 4 * (t + 1)]
                st = cst[:, P + 4 * N_BOX_TILES + 2 * t : P + 4 * N_BOX_TILES + 2 * (t + 1)]
                x1, x2, y1, y2 = (ed[:, i : i + 1] for i in range(4))

                def member(lo, hi, tag):
                    """m[n,w] = (w >= lo) & (w <= hi)"""
                    g2 = smallp.tile([P, P], F32, tag=tag + "g2")
                    nc.vector.tensor_scalar(
                        g2[:], iota_f, hi, None, op0=mybir.AluOpType.is_le
                    )
                    m = boxp.tile([P, P], F32, tag=tag + "m")
                    nc.vector.scalar_tensor_tensor(
                        m[:], iota_f, lo, g2[:],
                        op0=mybir.AluOpType.is_ge, op1=mybir.AluOpType.mult,
                    )
                    return m

                cols_val[t] = member(x1, x2, f"c{t}")
                rows_raw.append(member(y1, y2, f"r{t}"))
                sel_tiles.append(st)

            # --- per-image mask -> flat [1, HW] bf16 on partition 0 via a
            # small HWDGE flatten on the scalar ring. Image 0 first so the
            # main stream unblocks early.
            mflat = []
            for j in range(B_SHARD):
                for t in range(N_BOX_TILES):
                    rs = boxp.tile([P, P], F32, tag=f"rs{t}_{j}")
                    nc.vector.tensor_scalar(
                        rs[:], rows_raw[t][:], sel_tiles[t][:, j : j + 1], None,
                        op0=mybir.AluOpType.mult,
                    )
                    rows_sel[j][t] = rs
                pm = ps_m.tile([P, W], F32)
                for t in range(N_BOX_TILES):
                    nc.tensor.matmul(
                        pm[:], rows_sel[j][t][:], cols_val[t][:],
                        start=(t == 0), stop=(t == N_BOX_TILES - 1),
                    )
                msb = maskp.tile([P, W], BF16, tag=f"msb{j}")
                nc.vector.tensor_scalar_min(msb[:], pm[:], 1.0)
                mf = maskp.tile([1, HW], BF16, tag=f"mf{j}")
                nc.scalar.dma_start(mf[:], msb[:])
                mflat.append(mf)

            # --- main stream: 8 chunks of [256, 4096], software-pipelined.
            # One 2 MiB load per chunk on the sync HWDGE ring (loads only:
            # no store waits can head-of-line-block the prefetch); one 2 MiB
            # store per chunk via SWDGE on the Pool queue (store waits only
            # block other stores). X/O tiles are [128, 2*CHUNK]: columns
            # 0:CHUNK = channels 0-127, CHUNK:2*CHUNK = channels 128-255.
            def emit_front(b, off, ln):
                csl = slice(off, off + ln)
                X = xp.tile([P, 2 * ln], BF16, tag="X")
                nc.sync.dma_start(
                    X[:], x[b, :, csl].rearrange("(h p) w -> p h w", h=2)
                )
                # channel sums: one long same-stationary PE streak
                pss, ads = [], []
                npair = max(1, ln // PAIR)
                plen = min(ln, PAIR)
                for sp in range(npair):
                    ps = ps_s.tile([1, PAIR], F32)
                    for h in range(plen // SUB):
                        ssl = slice((sp * 2 + h) * SUB, (sp * 2 + h + 1) * SUB)
                        bsl = slice(
                            ln + (sp * 2 + h) * SUB, ln + (sp * 2 + h + 1) * SUB
                        )
                        hsl = slice(h * SUB, (h + 1) * SUB)
                        nc.tensor.matmul(
                            ps[:, hsl], wsum[:], X[:, ssl], start=True, stop=False
                        )
                        nc.tensor.matmul(
                            ps[:, hsl], wsum[:], X[:, bsl], start=False, stop=True
                        )
                    pss.append(ps)
                    # masked means (frees the ps slot for pair sp+2)
                    ad = adp.tile([1, PAIR], BF16, tag="ad")
                    moff = off + sp * PAIR
                    nc.vector.tensor_mul(
                        ad[:, 0:plen], ps[:, 0:plen], mflat[b][0:1, moff : moff + plen]
                    )
                    ads.append(ad)
                # broadcasts: second same-stationary PE streak; ACT converts
                pbs = pbsp.tile([P, ln], BF16, tag="pbs")
                for sp in range(npair):
                    for h in range(plen // SUB):
                        pb = ps_b.tile([P, SUB], F32)
                        nc.tensor.matmul(
                            pb[:], ones1[:], ads[sp][:, h * SUB : (h + 1) * SUB],
                            start=True, stop=True,
                        )
                        psl = slice((sp * 2 + h) * SUB, (sp * 2 + h + 1) * SUB)
                        nc.scalar.copy(pbs[:, psl], pb[:])
                return (b, off, ln, X, pbs)

            def emit_back(st):
                b, off, ln, X, pbs = st
                csl = slice(off, off + ln)
                half = ln // 2
                # adds split 3:1 between DVE and the warmed Pool engine so
                # the DVE queue stops pacing the chunk period
                O = outp.tile([P, 2 * ln], BF16, tag="O")
                nc.vector.tensor_add(O[:, 0:ln], X[:, 0:ln], pbs[:])
                nc.vector.tensor_add(
                    O[:, ln : ln + half], X[:, ln : ln + half], pbs[:, 0:half]
                )
                nc.gpsimd.tensor_add(
                    O[:, ln + half : 2 * ln], X[:, ln + half : 2 * ln],
                    pbs[:, half:ln],
                )
                nc.gpsimd.dma_start(
                    out[b, :, csl].rearrange("(h p) w -> p h w", h=2), O[:]
                )

            # pipeline priming: small leading chunks retire ~4x sooner, so
            # stores start flowing while the big prefetched loads are still
            # in flight (otherwise DMA idles ~18us waiting on chunk 0's
            # cold-pipeline round trip)
            sched = [(0, o, l) for o, l in
                     [(0, 1024), (1024, 1024), (2048, 2048), (4096, 4096),
                      (8192, 4096), (12288, 4096)]]
            sched += [(1, o, l) for o, l in
                      [(0, 4096), (4096, 4096), (8192, 4096), (12288, 2048),
                       (14336, 1024), (15360, 1024)]]
            prev = None
            for b, off, ln in sched:
                cur = emit_front(b, off, ln)
                if prev is not None:
                    emit_back(prev)
                prev = cur
            emit_back(prev)

    return nc


def _host_prep(x, bboxes, batch_idx):
    """Shard inputs; cast x to bf16; build the packed const array."""
    x = (
        np.ascontiguousarray(np.asarray(x, dtype=np.float32))
        .reshape(B, C, HW)
        .astype(ml_dtypes.bfloat16)
    )
    bboxes = np.asarray(bboxes, dtype=np.float32)
    batch_idx = np.asarray(batch_idx).astype(np.int64)

    # integer box edges, exact reference semantics (trunc toward zero,
    # clamp, validity); invalid or padded boxes -> x1=1, x2=0 (empty mask)
    xc, yc, bw, bh = (bboxes[:, i] for i in range(4))
    x1 = np.maximum(0, np.trunc((xc - bw / 2) * W)).astype(np.float32)
    y1 = np.maximum(0, np.trunc((yc - bh / 2) * H)).astype(np.float32)
    x2 = np.minimum(W - 1, np.trunc((xc + bw / 2) * W)).astype(np.float32)
    y2 = np.minimum(H - 1, np.trunc((yc + bh / 2) * H)).astype(np.float32)
    invalid = ~((x2 > x1) & (y2 > y1))
    x1, x2 = np.where(invalid, 1.0, x1), np.where(invalid, 0.0, x2)
    edges_pad = np.zeros((NBOX_PAD, 4), dtype=np.float32)
    edges_pad[:, 0] = 1.0  # padded boxes rasterize to nothing
    edges_pad[:N_BOX] = np.stack([x1, x2, y1, y2], axis=1)
    # [128, 12]: partition p, tile t -> box t*128+p
    boxes_cols = edges_pad.reshape(N_BOX_TILES, P, 4).transpose(1, 0, 2).reshape(P, -1)
    iota = np.broadcast_to(np.arange(P, dtype=np.float32), (P, P))

    in_maps = []
    for i in range(N_CORES):
        sel_i = np.zeros((NBOX_PAD, 2), dtype=np.float32)
        for j in range(B_SHARD):
            sel_i[:N_BOX, j] = (batch_idx == (i * B_SHARD + j)).astype(np.float32)
        sel_cols = sel_i.reshape(N_BOX_TILES, P, 2).transpose(1, 0, 2).reshape(P, -1)
        consts = np.concatenate([iota, boxes_cols, sel_cols], axis=1).astype(np.float32)
        in_maps.append(
            {
                "x": np.ascontiguousarray(x[i * B_SHARD : (i + 1) * B_SHARD]),
                "consts": np.ascontiguousarray(consts),
            }
        )
    return in_maps


def kernel(x, bboxes, batch_idx):
    in_maps = _host_prep(x, bboxes, batch_idx)
    nc = build_nc()
    nc.finalize()
    res = bass_utils.run_bass_kernel_spmd(nc, in_maps, core_ids=list(range(N_CORES)))
    shards = [
        np.asarray(res.results[i]["out"]).astype(np.float32) for i in range(N_CORES)
    ]
    return np.concatenate(shards, axis=0).reshape(B, C, H, W)


if __name__ == "__main__":
    nc = build_nc()
    nc.finalize()
    print("built ok:", len(nc.inst_map), "instructions")


# revision 27
# speedup vs baseline: 1.2183x; 1.2183x over previous
"""AssistedExcitation Trainium2 kernel.

out[b,c,h,w] = x[b,c,h,w] + bbox_mask[b,h,w] * mean_c(x[b,:,h,w])

Data-parallel over 8 NeuronCores: 2 images per core, no collectives.
HBM I/O in bf16 (rel-err budget 2e-2 >> bf16 rounding ~3e-3): halves
DMA traffic vs f32 -> ~94us/core roofline at 358 GB/s.

Per core, per [256, 4096] chunk (channel halves A/B on partitions):
  - channel sums via matmul with a 1/256 bf16 column into [1,1024]
    PSUM pair tiles,
  - DVE mul with the flat [1,HW] bf16 bbox mask -> masked means (ad),
  - K=1 broadcast matmuls spread ad across 128 partitions (PSUM),
    grouped after all sums so the PE runs long same-stationary streaks
    (TRN2 PE p-state: full 2.4 GHz only after ~3us continuous busy),
  - ACT copies PSUM->SBUF bf16, DVE does two fused 4096-wide bf16 adds
    (all-SBUF bf16 step-1 => 2x DVE packing mode),
  - chunk pipeline is software-staggered: adds/stores of chunk c-1 are
    emitted after the front half of chunk c, keeping every in-order
    engine queue free of cross-engine round-trip stalls.
Preamble (box rasterization) reads one packed [128,146] const DMA on
the scalar ring so it never queues behind 1 MiB x loads on sync.
"""

import sys

sys.path.insert(0, "/opt/trn_rl_repo")

import ml_dtypes
import numpy as np

import concourse.bacc as bacc
import concourse.bass as bass
import concourse.mybir as mybir
import concourse.tile as tile
from concourse import bass_utils

# Problem constants (hardcoded per harness contract)
B, C, H, W = 16, 256, 128, 128
N_BOX = 320
N_CORES = 8
B_SHARD = B // N_CORES  # 2 images per core
HW = H * W  # 16384
P = 128  # partitions
CHUNK = 4096  # free-dim elements per x tile (32 rows of the image)
N_CHUNK = HW // CHUNK  # 4
SUB = 512  # matmul moving free-dim (one PSUM bank of f32)
PAIR = 2 * SUB  # 1024: one [1, PAIR] PSUM sum tile = 2 banks
N_PAIR = CHUNK // PAIR  # 4
NBOX_PAD = 384  # 320 boxes padded to 3 tiles of 128
N_BOX_TILES = NBOX_PAD // P  # 3
ALPHA = 1.0
# packed const layout: [iota(128) | int box edges x1,x2,y1,y2 (3*4) | sel(3*2)]
CONST_COLS = P + 4 * N_BOX_TILES + 2 * N_BOX_TILES  # 146

F32 = mybir.dt.float32
BF16 = mybir.dt.bfloat16


def build_nc():
    """Build the per-core Bass graph (SPMD: same graph on all 8 cores)."""
    nc = bacc.Bacc(None, target_bir_lowering=False)

    x = nc.declare_dram_parameter("x", [B_SHARD, C, HW], BF16, isOutput=False)
    consts = nc.declare_dram_parameter("consts", [P, CONST_COLS], F32, isOutput=False)
    out = nc.declare_dram_parameter("out", [B_SHARD, C, HW], BF16, isOutput=True)

    with tile.TileContext(nc) as tc:
        with (
            tc.tile_pool(name="const", bufs=1) as constp,
            tc.tile_pool(name="boxp", bufs=1) as boxp,
            tc.tile_pool(name="maskp", bufs=1) as maskp,
            tc.tile_pool(name="xp", bufs=4) as xp,
            tc.tile_pool(name="outp", bufs=2) as outp,
            tc.tile_pool(name="pbsp", bufs=2) as pbsp,
            tc.tile_pool(name="adp", bufs=2) as adp,
            tc.tile_pool(name="smallp", bufs=2) as smallp,
            tc.tile_pool(name="ps_s", bufs=2, space=bass.MemorySpace.PSUM) as ps_s,
            tc.tile_pool(name="ps_b", bufs=3, space=bass.MemorySpace.PSUM) as ps_b,
            tc.tile_pool(name="ps_m", bufs=1, space=bass.MemorySpace.PSUM) as ps_m,
        ):
            # --- constants: one small DMA on the scalar ring ---
            cst = constp.tile([P, CONST_COLS], F32)
            nc.scalar.dma_start(cst[:], consts[:])
            iota_f = cst[:, 0:P]
            wsum = constp.tile([P, 1], BF16)  # 1/C column -> channel mean
            nc.vector.memset(wsum[:], ALPHA / C)
            ones1 = constp.tile([1, P], BF16)  # K=1 broadcast row
            nc.vector.memset(ones1[:], 1.0)
            # warm the Pool tensor_tensor ucode now: its first call pays a
            # ~6us IRAM load, which must not land on the steady-state path
            warm = constp.tile([1, 4], BF16)
            nc.gpsimd.memset(warm[:], 0.0)
            nc.gpsimd.tensor_add(warm[:], warm[:], warm[:])

            # --- box rasterization (tiny): integer box edges + validity are
            # host-precomputed (exact reference trunc/clamp semantics;
            # invalid boxes get x1=1,x2=0 -> empty). Device only compares:
            # cols[n,w] = (w >= x1) & (w <= x2).
            rows_sel = [[None] * N_BOX_TILES for _ in range(B_SHARD)]
            cols_val = [None] * N_BOX_TILES
            rows_raw, sel_tiles = [], []
            for t in range(N_BOX_TILES):
                ed = cst[:, P + 4 * t : P + 4 * (t + 1)]
                st = cst[:, P + 4 * N_BOX_TILES + 2 * t : P + 4 * N_BOX_TILES + 2 * (t + 1)]
                x1, x2, y1, y2 = (ed[:, i : i + 1] for i in range(4))

                def member(lo, hi, tag):
                    """m[n,w] = (w >= lo) & (w <= hi)"""
                    g2 = smallp.tile([P, P], F32, tag=tag + "g2")
                    nc.vector.tensor_scalar(
                        g2[:], iota_f, hi, None, op0=mybir.AluOpType.is_le
                    )
                    m = boxp.tile([P, P], F32, tag=tag + "m")
                    nc.vector.scalar_tensor_tensor(
                        m[:], iota_f, lo, g2[:],
                        op0=mybir.AluOpType.is_ge, op1=mybir.AluOpType.mult,
                    )
                    return m

                cols_val[t] = member(x1, x2, f"c{t}")
                rows_raw.append(member(y1, y2, f"r{t}"))
                sel_tiles.append(st)

            # --- per-image mask -> flat [1, HW] bf16 on partition 0 via a
            # small HWDGE flatten on the scalar ring. Image 0 first so the
            # main stream unblocks early.
            mflat = []
            for j in range(B_SHARD):
                for t in range(N_BOX_TILES):
                    rs = boxp.tile([P, P], F32, tag=f"rs{t}_{j}")
                    nc.vector.tensor_scalar(
                        rs[:], rows_raw[t][:], sel_tiles[t][:, j : j + 1], None,
                        op0=mybir.AluOpType.mult,
                    )
                    rows_sel[j][t] = rs
                pm = ps_m.tile([P, W], F32)
                for t in range(N_BOX_TILES):
                    nc.tensor.matmul(
                        pm[:], rows_sel[j][t][:], cols_val[t][:],
                        start=(t == 0), stop=(t == N_BOX_TILES - 1),
                    )
                msb = maskp.tile([P, W], BF16, tag=f"msb{j}")
                nc.vector.tensor_scalar_min(msb[:], pm[:], 1.0)
                mf = maskp.tile([1, HW], BF16, tag=f"mf{j}")
                nc.scalar.dma_start(mf[:], msb[:])
                mflat.append(mf)

            # --- main stream: 8 chunks of [256, 4096], software-pipelined.
            # One 2 MiB load per chunk on the sync HWDGE ring (loads only:
            # no store waits can head-of-line-block the prefetch); one 2 MiB
            # store per chunk via SWDGE on the Pool queue (store waits only
            # block other stores). X/O tiles are [128, 2*CHUNK]: columns
            # 0:CHUNK = channels 0-127, CHUNK:2*CHUNK = channels 128-255.
            def emit_front(b, off, ln):
                csl = slice(off, off + ln)
                X = xp.tile([P, 2 * ln], BF16, tag="X")
                nc.sync.dma_start(
                    X[:], x[b, :, csl].rearrange("(h p) w -> p h w", h=2)
                )
                # channel sums: one long same-stationary PE streak
                pss, ads = [], []
                npair = max(1, ln // PAIR)
                plen = min(ln, PAIR)
                for sp in range(npair):
                    ps = ps_s.tile([1, PAIR], F32)
                    for h in range(plen // SUB):
                        ssl = slice((sp * 2 + h) * SUB, (sp * 2 + h + 1) * SUB)
                        bsl = slice(
                            ln + (sp * 2 + h) * SUB, ln + (sp * 2 + h + 1) * SUB
                        )
                        hsl = slice(h * SUB, (h + 1) * SUB)
                        nc.tensor.matmul(
                            ps[:, hsl], wsum[:], X[:, ssl], start=True, stop=False
                        )
                        nc.tensor.matmul(
                            ps[:, hsl], wsum[:], X[:, bsl], start=False, stop=True
                        )
                    pss.append(ps)
                    # masked means (frees the ps slot for pair sp+2)
                    ad = adp.tile([1, PAIR], BF16, tag="ad")
                    moff = off + sp * PAIR
                    nc.vector.tensor_mul(
                        ad[:, 0:plen], ps[:, 0:plen], mflat[b][0:1, moff : moff + plen]
                    )
                    ads.append(ad)
                # broadcasts: second same-stationary PE streak; ACT converts
                pbs = pbsp.tile([P, ln], BF16, tag="pbs")
                for sp in range(npair):
                    for h in range(plen // SUB):
                        pb = ps_b.tile([P, SUB], F32)
                        nc.tensor.matmul(
                            pb[:], ones1[:], ads[sp][:, h * SUB : (h + 1) * SUB],
                            start=True, stop=True,
                        )
                        psl = slice((sp * 2 + h) * SUB, (sp * 2 + h + 1) * SUB)
                        nc.scalar.copy(pbs[:, psl], pb[:])
                return (b, off, ln, X, pbs)

            def emit_back(st):
                b, off, ln, X, pbs = st
                csl = slice(off, off + ln)
                O = outp.tile([P, 2 * ln], BF16, tag="O")
                nc.vector.tensor_add(O[:, 0:ln], X[:, 0:ln], pbs[:])
                nc.vector.tensor_add(O[:, ln : 2 * ln], X[:, ln : 2 * ln], pbs[:])
                nc.gpsimd.dma_start(
                    out[b, :, csl].rearrange("(h p) w -> p h w", h=2), O[:]
                )

            # pipeline priming: small leading chunks retire ~4x sooner, so
            # stores start flowing while the big prefetched loads are still
            # in flight (otherwise DMA idles ~18us waiting on chunk 0's
            # cold-pipeline round trip)
            sched = [(0, o, l) for o, l in
                     [(0, 1024), (1024, 1024), (2048, 2048), (4096, 4096),
                      (8192, 4096), (12288, 4096)]]
            sched += [(1, o, l) for o, l in
                      [(0, 4096), (4096, 4096), (8192, 4096), (12288, 2048),
                       (14336, 1024), (15360, 1024)]]
            prev = None
            for b, off, ln in sched:
                cur = emit_front(b, off, ln)
                if prev is not None:
                    emit_back(prev)
                prev = cur
            emit_back(prev)

    return nc


def _host_prep(x, bboxes, batch_idx):
    """Shard inputs; cast x to bf16; build the packed const array."""
    x = (
        np.ascontiguousarray(np.asarray(x, dtype=np.float32))
        .reshape(B, C, HW)
        .astype(ml_dtypes.bfloat16)
    )
    bboxes = np.asarray(bboxes, dtype=np.float32)
    batch_idx = np.asarray(batch_idx).astype(np.int64)

    # integer box edges, exact reference semantics (trunc toward zero,
    # clamp, validity); invalid or padded boxes -> x1=1, x2=0 (empty mask)
    xc, yc, bw, bh = (bboxes[:, i] for i in range(4))
    x1 = np.maximum(0, np.trunc((xc - bw / 2) * W)).astype(np.float32)
    y1 = np.maximum(0, np.trunc((yc - bh / 2) * H)).astype(np.float32)
    x2 = np.minimum(W - 1, np.trunc((xc + bw / 2) * W)).astype(np.float32)
    y2 = np.minimum(H - 1, np.trunc((yc + bh / 2) * H)).astype(np.float32)
    invalid = ~((x2 > x1) & (y2 > y1))
    x1, x2 = np.where(invalid, 1.0, x1), np.where(invalid, 0.0, x2)
    edges_pad = np.zeros((NBOX_PAD, 4), dtype=np.float32)
    edges_pad[:, 0] = 1.0  # padded boxes rasterize to nothing
    edges_pad[:N_BOX] = np.stack([x1, x2, y1, y2], axis=1)
    # [128, 12]: partition p, tile t -> box t*128+p
    boxes_cols = edges_pad.reshape(N_BOX_TILES, P, 4).transpose(1, 0, 2).reshape(P, -1)
    iota = np.broadcast_to(np.arange(P, dtype=np.float32), (P, P))

    in_maps = []
    for i in range(N_CORES):
        sel_i = np.zeros((NBOX_PAD, 2), dtype=np.float32)
        for j in range(B_SHARD):
            sel_i[:N_BOX, j] = (batch_idx == (i * B_SHARD + j)).astype(np.float32)
        sel_cols = sel_i.reshape(N_BOX_TILES, P, 2).transpose(1, 0, 2).reshape(P, -1)
        consts = np.concatenate([iota, boxes_cols, sel_cols], axis=1).astype(np.float32)
        in_maps.append(
            {
                "x": np.ascontiguousarray(x[i * B_SHARD : (i + 1) * B_SHARD]),
                "consts": np.ascontiguousarray(consts),
            }
        )
    return in_maps


def kernel(x, bboxes, batch_idx):
    in_maps = _host_prep(x, bboxes, batch_idx)
    nc = build_nc()
    nc.finalize()
    res = bass_utils.run_bass_kernel_spmd(nc, in_maps, core_ids=list(range(N_CORES)))
    shards = [
        np.asarray(res.results[i]["out"]).astype(np.float32) for i in range(N_CORES)
    ]
    return np.concatenate(shards, axis=0).reshape(B, C, H, W)


if __name__ == "__main__":
    nc = build_nc()
    nc.finalize()
    print("built ok:", len(nc.inst_map), "instructions")
